# revision 1
# baseline (speedup 1.0000x reference)
"""Trainium2 Bass kernel for CIN (Compressed Interaction Network) forward.

Reference computation (per batch b, per dim d, with x = inputs[b, :, d], F=32):
  z0[(h,m)] = x[h]*x[m]                    (1024-vector)
  y0 = relu(W0 @ z0 + b0)                  (128)
  h1 = y0[:64]; f0 = y0[64:]
  z1[(g,m)] = h1[g]*x[m]                   (2048-vector)
  f1 = relu(W1 @ z1 + b1)                  (128)
  out[b, 0:64]  = sum_d f0
  out[b, 64:192] = sum_d f1

Strategy: pure data parallel over 8 cores (256 batch each). Per core the
(b, d) pairs form 16384 GEMM columns, processed in 16 chunks of 1024. The
outer-product features z are built on the Vector engine by a custom DVE op
fusing the 32-lane SHUFFLE front-end (per-quadrant lane broadcast; mask
state set via stream_shuffle, ordering enforced with pad-column dep chains)
with an elementwise multiply. GEMMs run on the Tensor engine in fp16.
relu + d-reduction are fused on the Scalar engine via accum_out. The final
(channel, batch) -> (batch, channel) transpose runs on the Tensor engine.
Chunks are software-pipelined one deep: layer-0 of chunk i+1 overlaps
layer-1 of chunk i so the Vector engine never waits on layer-0 outputs.
"""

import sys

sys.path.insert(0, "/opt/trn_rl_repo")

import numpy as np

import concourse.bass as bass
import concourse.mybir as mybir
import concourse.tile as tile
from concourse.tile import add_dep_helper
from concourse import bacc
from concourse.bass_utils import run_bass_kernel_spmd
from concourse.masks import make_identity

import concourse.dve_ops as dve_ops
from concourse.dve_ops import DveOp, OPS, CUSTOM_DVE_SPECS, _SUB_OPCODE_FOR_NAME
from concourse.dve_spec import Spec, Src0, Src1, lower, _has_src1
from concourse.dve_uop import (
    DveOpSpec,
    OpConfig,
    TransposeMode,
    ENABLE,
    UopConfig,
    InpSel,
    OutSel,
    OutPath,
    AluOp,
    AluInp,
    DelayInp,
    Trigger,
)

# ---- problem constants (hardcoded per contract) ---------------------------- #
B = 2048
F = 32  # field size (channels in)
D = 64  # embedding dim
O0 = 128  # layer-0 out channels
O1 = 128  # layer-1 out channels
H1 = 64  # split half fed to layer 1
NCORES = 8
BC = B // NCORES  # batch per core
NCHUNK = 1024  # GEMM columns per chunk (16 batch x 64 d)
BPC = NCHUNK // D  # batch elems per chunk
NCHUNKS = BC * D // NCHUNK
L0C = 5  # layer-0 z chunks (symmetric cover: difference classes 0..16)
L0_SHIFT = (0, 4, 8, 12, 16)  # mask shift per layer-0 chunk
L1C = 16  # layer-1 z chunks (2048 rows / 128)
PAD = 2  # pad columns used for mask-ordering dependency chains
MMF = 512  # matmul free-dim per instruction
DT = mybir.dt.float16
FP32 = mybir.dt.float32
W = NCHUNK + PAD


def _mul_2x_uop():
    """2X_1PORT program for out = src0 * src1 on packed 16-bit data (mirror of
    the stock TENSOR_TENSOR 2X_1P program with the ALU op fixed to MULTIPLY).
    NOTE: kept for reference/experiments — combining 2x with the SHUFFLE
    front-end faults on hardware, so SHUF_MUL runs with perf_max=0 (1x)."""
    u = UopConfig()
    u.enable_input(InpSel.SRC_0, 0)
    u.enable_input(InpSel.SRC_1, 1)
    u.enable_input(InpSel.SRC_0_HI, 2)
    u.enable_input(InpSel.SRC_1_HI, 3)
    u.require_inp0 = ENABLE
    u.require_inp1 = ENABLE
    u.trigger = (Trigger.SRC_TENSOR_DONE, Trigger.NONE, Trigger.NONE)
    u.enable_output(OutSel.ALU_OUT, OutPath.WR0_LO)
    u.enable_output(OutSel.DELAY_0, OutPath.WR0_HI)
    b0 = u.datapath_config[0]
    b0.enable_alu(AluOp.MULTIPLY, AluInp.PREV_ALU_OUT, AluInp.PREV_DELAY_0)
    b0.enable_delay_from_src(DelayInp.PREV_DELAY, 1)
    b0.enable_delay_from_src(DelayInp.PREV_DELAY, 2)
    b1 = u.datapath_config[1]
    b1.enable_alu(AluOp.MULTIPLY, AluInp.PREV_DELAY_1, AluInp.PREV_DELAY_2)
    b1.enable_delay_from_src(DelayInp.PREV_ALU_OUT, 0)
    b2 = u.datapath_config[2]
    b2.enable_alu(AluOp.BYPASS, AluInp.PREV_DELAY_0, AluInp.PREV_DELAY_0)
    b2.enable_delay_from_src(DelayInp.PREV_ALU_OUT, 0)
    for k in range(3, 8):
        u.datapath_config[k].pass_through_alu()
        u.datapath_config[k].pass_through_delay(0)
    return u


class _ShuffleMulOp(DveOp):
    """Elementwise multiply with the SHUFFLE front-end applied to src0.

    out[32q+l, n] = src0[32q + mask[l], n] * src1[32q+l, n]
    mask = the DVE MaskSelect state (set by a preceding stream_shuffle).
    """

    def compile(self, ver):
        key = (self.name, ver)
        cache = dve_ops._COMPILE_CACHE
        if (r := cache.get(key)) is not None:
            return r
        result = DveOpSpec(
            name=self.name,
            opcode=dve_ops.get_dve_sub_opcode(self.name),
            uops=lower(self.spec, ver=ver),
            rd1_en=_has_src1(self.spec),
            op=OpConfig(transpose_mode=TransposeMode.SHUFFLE, mask_enable=ENABLE),
        )
        cache[key] = result
        return result


def _register_shuf_mul():
    if "SHUF_MUL_ANT" in _SUB_OPCODE_FOR_NAME:
        return next(op for op in OPS if op.name == "SHUF_MUL_ANT")
    op = _ShuffleMulOp(
        "SHUF_MUL_ANT",
        Spec(body=Src0 * Src1, reference=lambda in0, in1, s0, s1, imm2: in0 * in1),
        subdim=False,
        uops_sha={},
    )
    OPS.append(op)
    CUSTOM_DVE_SPECS[op.name] = op.spec
    _SUB_OPCODE_FOR_NAME[op.name] = max(_SUB_OPCODE_FOR_NAME.values()) + 1
    assert _SUB_OPCODE_FOR_NAME[op.name] < 0x20
    return op


# ---- host-side data prep --------------------------------------------------- #
#
# Row maps. Layer 0 exploits z0 symmetry (x[h]*x[m] = x[m]*x[h]): unordered
# pairs {h, m} are classified by circular difference delta = (h-m) mod 32,
# delta in 0..16 (528 pairs total). Chunk c (c in 0..4), quadrant q covers
# class delta = L0_SHIFT[c] + q at all positions:
#   z-row 32q+l of chunk c = x[(l + L0_SHIFT[c] + q) % 32] * x[l]
# produced with lane-varying mask mask_c[l] = (l + L0_SHIFT[c]) % 32 from
#   xa[32q + m] = x[m],  xr0[32q + j] = x[(j + q) % 32]
# The symmetrized weight (W0[o,h,m] + W0[o,m,h]) is assigned to the single
# slot covering {h,m}; duplicate slots (delta 17..19, and half of delta=16)
# carry zero weights.
# Layer 1, chunk c (c in 0..15): z-row 32q+l holds pair
#   (g, m) = (32*(q%2) + l, (c + 16*(q//2)) % 32)
# produced with constant mask c from
#   xr1[32q + j] = x[(j + 16*(q//2)) % 32],  y2[32q + l] = y[32*(q%2) + l]


def _prep_weights(W0, b0, W1, b1):
    w0 = W0.reshape(O0, F, F)  # [o, h, m]
    w0sym = w0 + w0.transpose(0, 2, 1)
    w0t = np.zeros((L0C, 128, O0), dtype=np.float16)
    for c in range(L0C):
        for q in range(4):
            delta = L0_SHIFT[c] + q
            if delta > 16:
                continue  # duplicate class, keep zero weights
            for l in range(32):
                if delta == 16 and l >= 16:
                    continue  # delta=16 pairs appear twice; keep first half
                h = (l + delta) % 32
                if delta == 0:
                    w0t[c, 32 * q + l, :] = w0[:, l, l].astype(np.float16)
                else:
                    w0t[c, 32 * q + l, :] = w0sym[:, h, l].astype(np.float16)
    w1 = W1.reshape(O1, H1, F)  # [o, g, m]
    w1t = np.empty((L1C, 128, O1), dtype=np.float16)
    for c in range(L1C):
        for q in range(4):
            m = (c + 16 * (q // 2)) % 32
            gbase = 32 * (q % 2)
            w1t[c, 32 * q : 32 * q + 32, :] = w1[:, gbase : gbase + 32, m].T.astype(
                np.float16
            )
    # [p, c, o] layout for contiguous per-partition DMA
    return (
        np.ascontiguousarray(w0t.transpose(1, 0, 2)),
        np.ascontiguousarray(w1t.transpose(1, 0, 2)),
        b0.astype(np.float32),
        b1.astype(np.float32),
    )


def _prep_inputs_core(x_core):
    """x_core: (BC, F, D) fp32 -> xa, xr0, xr1 tiles (NCHUNKS, 128, W) fp16."""
    xcols = (
        x_core.reshape(NCHUNKS, BPC, F, D)
        .transpose(2, 0, 1, 3)
        .reshape(F, NCHUNKS, NCHUNK)
        .astype(np.float16)
    )
    xa = np.zeros((NCHUNKS, 128, W), dtype=np.float16)
    xr0 = np.zeros((NCHUNKS, 128, W), dtype=np.float16)
    xr1 = np.zeros((NCHUNKS, 128, W), dtype=np.float16)
    for q in range(4):
        rows = slice(32 * q, 32 * q + 32)
        xa[:, rows, :NCHUNK] = xcols.transpose(1, 0, 2)
        r0 = np.roll(np.arange(F), -q)  # j -> (j + q) % 32
        xr0[:, rows, :NCHUNK] = xcols[r0].transpose(1, 0, 2)
        r1 = np.roll(np.arange(F), -16 * (q // 2))
        xr1[:, rows, :NCHUNK] = xcols[r1].transpose(1, 0, 2)
    return xa, xr0, xr1


# ---- kernel build ---------------------------------------------------------- #

_NC_CACHE = {}


def _build(op):
    nc = bacc.Bacc("TRN2", target_bir_lowering=False, debug=False)

    xa_d = nc.dram_tensor("xa", [NCHUNKS, 128, W], DT, kind="ExternalInput")
    xr0_d = nc.dram_tensor("xr0", [NCHUNKS, 128, W], DT, kind="ExternalInput")
    xr1_d = nc.dram_tensor("xr1", [NCHUNKS, 128, W], DT, kind="ExternalInput")
    w0t_d = nc.dram_tensor("w0t", [128, L0C, O0], DT, kind="ExternalInput")
    w1t_d = nc.dram_tensor("w1t", [128, L1C, O1], DT, kind="ExternalInput")
    b0_d = nc.dram_tensor("b0", [O0, 1], FP32, kind="ExternalInput")
    b1_d = nc.dram_tensor("b1", [O1, 1], FP32, kind="ExternalInput")
    out_d = nc.dram_tensor("out", [BC, 192], FP32, kind="ExternalOutput")

    with tile.TileContext(nc) as tc:
        with (
            tc.tile_pool(name="const", bufs=1) as cpool,
            tc.tile_pool(name="xin", bufs=6) as xpool,
            tc.tile_pool(name="z", bufs=6) as zpool,
            tc.tile_pool(name="ytmp", bufs=4) as ypool,
            tc.tile_pool(name="scratch", bufs=2) as spool,
            tc.tile_pool(name="psum", bufs=2, space="PSUM") as pspool,
        ):
            # resident weights, biases, accumulators, identity
            w0t = cpool.tile([128, L0C, O0], DT, tag="w0t")
            w1t = cpool.tile([128, L1C, O1], DT, tag="w1t")
            nc.sync.dma_start(w0t[:], w0t_d.ap())
            nc.sync.dma_start(w1t[:], w1t_d.ap())
            b0t = cpool.tile([O0, 1], FP32, tag="b0")
            b1t = cpool.tile([O1, 1], FP32, tag="b1")
            nc.sync.dma_start(b0t[:], b0_d.ap())
            nc.sync.dma_start(b1t[:], b1_d.ap())
            ident = cpool.tile([128, 128], FP32, tag="ident")
            make_identity(nc, ident[:])
            r0all = cpool.tile([128, BC], FP32, tag="r0all")  # rows 64:128 used
            r1all = cpool.tile([128, BC], FP32, tag="r1all")

            msink = cpool.tile([128, 2], DT, tag="msink")
            state = {"prev_z": None}

            def set_mask(mask):
                """Load MaskSelect. Reads the previous shuf_mul's pad column
                (so it runs after it); the following shuf_muls get explicit
                dep edges back to this instruction."""
                src = (
                    state["prev_z"][:, NCHUNK : NCHUNK + 1]
                    if state["prev_z"] is not None
                    else msink[:, 0:1]
                )
                return nc.vector.stream_shuffle(msink[:, 1:2], src, mask)

            def shuf_mul(ml, src0, src1, wt, c, ps, nchunks_c):
                z = zpool.tile([128, W], DT, tag="z")
                bi = nc.vector._custom_dve(op, out=z[:], in0=src0[:], in1=src1[:])
                add_dep_helper(bi.ins, ml.ins, sync=False, reason="mask state order")
                state["prev_z"] = z
                for s in range(NCHUNK // MMF):
                    nc.tensor.matmul(
                        ps[:, s * MMF : (s + 1) * MMF],
                        wt[:, c],
                        z[:, s * MMF : (s + 1) * MMF],
                        start=(c == 0),
                        stop=(c == nchunks_c - 1),
                    )

            chunks = {}

            def emit_l0_pair(p):
                cs = [2 * p, 2 * p + 1]
                tiles = []
                for i in cs:
                    xa = xpool.tile([128, W], DT, tag="xa")
                    xr0 = xpool.tile([128, W], DT, tag="xr0")
                    nc.sync.dma_start(xa[:], xa_d.ap()[i])
                    nc.sync.dma_start(xr0[:], xr0_d.ap()[i])
                    ps0 = pspool.tile([128, NCHUNK], FP32, tag="ps0")
                    tiles.append({"xa": xa, "xr0": xr0, "ps0": ps0})
                for c in range(L0C):
                    ml = set_mask([(l + L0_SHIFT[c]) % 32 for l in range(32)])
                    for t in tiles:
                        shuf_mul(ml, t["xr0"], t["xa"], w0t, c, t["ps0"], L0C)
                for i, t in zip(cs, tiles):
                    # xr1 is only needed for this chunk's layer-1 (one pair
                    # later) — load it off the startup critical path
                    xr1 = xpool.tile([128, W], DT, tag="xr1")
                    nc.sync.dma_start(xr1[:], xr1_d.ap()[i])
                    t["xr1"] = xr1
                    ps0 = t["ps0"]
                    y2 = ypool.tile([128, W], DT, tag="y2")
                    nc.scalar.activation(
                        y2[:H1, :NCHUNK],
                        ps0[:H1],
                        mybir.ActivationFunctionType.Relu,
                        bias=b0t[:H1],
                    )
                    # stage relu(f0) to SBUF in one op so ps0 frees quickly;
                    # accumulate the d-reduction from SBUF afterwards
                    f0s = ypool.tile([128, NCHUNK], DT, tag="f0s")
                    nc.scalar.activation(
                        f0s[H1:128],
                        ps0[H1:128],
                        mybir.ActivationFunctionType.Relu,
                        bias=b0t[H1:128],
                    )
                    scratch0 = spool.tile([128, D], DT, tag="scr0")
                    for b in range(BPC):
                        col = i * BPC + b
                        nc.scalar.activation(
                            scratch0[H1:128],
                            f0s[H1:128, b * D : (b + 1) * D],
                            mybir.ActivationFunctionType.Relu,
                            bias=0.0,
                            accum_out=r0all[H1:128, col : col + 1],
                        )
                    chunks[i] = {"xr1": t["xr1"], "y2": y2}

            def emit_l1_pair(p):
                cs = [2 * p, 2 * p + 1]
                sts = [chunks.pop(i) for i in cs]
                pss = [
                    pspool.tile([128, NCHUNK], FP32, tag="ps1", name=f"ps1_{k}")
                    for k in range(len(cs))
                ]
                for c in range(L1C):
                    ml = set_mask([c] * 32)
                    for st, ps1 in zip(sts, pss):
                        shuf_mul(ml, st["xr1"], st["y2"], w1t, c, ps1, L1C)
                scratch = spool.tile([128, D], DT, tag="scr")
                for i, ps1 in zip(cs, pss):
                    f1s = ypool.tile([128, NCHUNK], DT, tag="f1s")
                    nc.scalar.activation(
                        f1s[:],
                        ps1[:],
                        mybir.ActivationFunctionType.Relu,
                        bias=b1t[:],
                    )
                    for b in range(BPC):
                        col = i * BPC + b
                        nc.scalar.activation(
                            scratch[:],
                            f1s[:, b * D : (b + 1) * D],
                            mybir.ActivationFunctionType.Relu,
                            bias=0.0,
                            accum_out=r1all[:, col : col + 1],
                        )

            NPAIRS = NCHUNKS // 2
            for p in range(NPAIRS + 1):
                if p < NPAIRS:
                    emit_l0_pair(p)
                if p >= 1:
                    emit_l1_pair(p - 1)
                if p < NPAIRS:
                    # duplicate Y into upper halves for this pair's layer-1,
                    # emitted after layer-1 of the previous pair.
                    for i in (2 * p, 2 * p + 1):
                        y2 = chunks[i]["y2"]
                        nc.vector.tensor_copy(
                            y2[H1:128, :NCHUNK], y2[:H1, :NCHUNK]
                        )

            # ---- final transpose (channel, batch) -> (batch, channel) ----
            outbuf = cpool.tile([128, 2, 192], FP32, tag="outbuf")
            for seg in range(2):
                cs = slice(seg * 128, (seg + 1) * 128)
                pt1 = pspool.tile([128, 128], FP32, tag="ps1")
                nc.tensor.matmul(
                    pt1[:], r1all[:, cs], ident[:], is_transpose=True
                )
                nc.scalar.activation(
                    outbuf[:, seg, H1:192],
                    pt1[:],
                    mybir.ActivationFunctionType.Copy,
                )
                pt0 = pspool.tile([128, 64], FP32, tag="ps0")
                nc.tensor.matmul(
                    pt0[:],
                    r0all[H1:128, cs],
                    ident[H1:128, H1:128],
                    is_transpose=True,
                )
                nc.scalar.activation(
                    outbuf[:, seg, 0:H1],
                    pt0[:],
                    mybir.ActivationFunctionType.Copy,
                )
            nc.sync.dma_start(
                out_d.ap().rearrange("(s b) c -> b s c", s=2), outbuf[:]
            )

    nc.compile()
    return nc


def _get_nc():
    if "nc" not in _NC_CACHE:
        op = _register_shuf_mul()
        _NC_CACHE["nc"] = _build(op)
    return _NC_CACHE["nc"]


def _make_in_maps(inputs, W0, b0, W1, b1):
    w0t, w1t, b0f, b1f = _prep_weights(W0, b0, W1, b1)
    in_maps = []
    for core in range(NCORES):
        xa, xr0, xr1 = _prep_inputs_core(inputs[core * BC : (core + 1) * BC])
        in_maps.append(
            {
                "xa": xa,
                "xr0": xr0,
                "xr1": xr1,
                "w0t": w0t,
                "w1t": w1t,
                "b0": b0f[:, None],
                "b1": b1f[:, None],
            }
        )
    return in_maps


def kernel(inputs, W0, b0, W1, b1):
    inputs = np.asarray(inputs, dtype=np.float32)
    nc = _get_nc()
    in_maps = _make_in_maps(
        inputs,
        np.asarray(W0, np.float32),
        np.asarray(b0, np.float32),
        np.asarray(W1, np.float32),
        np.asarray(b1, np.float32),
    )
    res = run_bass_kernel_spmd(nc, in_maps, core_ids=list(range(NCORES)))
    out = np.concatenate([res.results[c]["out"] for c in range(NCORES)], axis=0)
    return out.astype(np.float32)


def _install_ntff_hook():
    """The container's antenv lacks axon_hooks; synthesize it around the
    injected libaxon_pjrt.so so run_bass_kernel_spmd(trace=True) works."""
    import types

    if "antenv.axon_hooks" in sys.modules:
        return
    sys.path.insert(0, "/root/.axon_site")
    from trn_agent_boot.trn_boot import _ntff_profile_via_ctypes

    hook = _ntff_profile_via_ctypes("/opt/axon/libaxon_pjrt.so")
    m = types.ModuleType("antenv.axon_hooks")
    m.get_axon_ntff_profile_hook = lambda: hook
    m.set_axon_ntff_profile_hook = lambda h: None
    sys.modules["antenv.axon_hooks"] = m


def profile_once(inputs_dict):
    """Run once with NTFF tracing; return exec_time_ns (core 0)."""
    _install_ntff_hook()
    nc = _get_nc()
    in_maps = _make_in_maps(
        np.asarray(inputs_dict["inputs"], np.float32),
        np.asarray(inputs_dict["W0"], np.float32),
        np.asarray(inputs_dict["b0"], np.float32),
        np.asarray(inputs_dict["W1"], np.float32),
        np.asarray(inputs_dict["b1"], np.float32),
    )
    res = run_bass_kernel_spmd(nc, in_maps, core_ids=list(range(NCORES)), trace=True)
    return res.exec_time_ns


if __name__ == "__main__":
    rng = np.random.default_rng(0)
    inputs = rng.standard_normal((B, F, D), dtype=np.float32)
    W0 = (rng.standard_normal((O0, F * F), dtype=np.float32) * 0.03).astype(np.float32)
    W1 = (rng.standard_normal((O1, H1 * F), dtype=np.float32) * 0.03).astype(np.float32)
    b0 = np.zeros(O0, np.float32)
    b1 = np.zeros(O1, np.float32)
    out = kernel(inputs=inputs, W0=W0, b0=b0, W1=W1, b1=b1)
    print("kernel out", out.shape, out.dtype, out[:2, :4])



# revision 2
# speedup vs baseline: 1.3378x; 1.3378x over previous
"""Trainium2 Bass kernel for CIN (Compressed Interaction Network) forward.

Reference computation (per batch b, per dim d, with x = inputs[b, :, d], F=32):
  z0[(h,m)] = x[h]*x[m]                    (1024-vector)
  y0 = relu(W0 @ z0 + b0)                  (128)
  h1 = y0[:64]; f0 = y0[64:]
  z1[(g,m)] = h1[g]*x[m]                   (2048-vector)
  f1 = relu(W1 @ z1 + b1)                  (128)
  out[b, 0:64]  = sum_d f0
  out[b, 64:192] = sum_d f1

Strategy: pure data parallel over 8 cores (256 batch each). Per core the
(b, d) pairs form 16384 GEMM columns, processed in 16 chunks of 1024.

The outer-product feature tiles z are built with PLAIN tensor_tensor
multiplies (which run in the DVE's 2x packed-fp16 mode, 2 elem/lane/cycle)
against host-precomputed partition-rotated / partition-broadcast operand
tiles streamed from HBM:
  z0 chunk c: xr0[:, c] * xa      (xr0 = 5 host-rotated variants of x)
  z1 chunk c: xb[:, c] * y2d      (xb = 16 host-broadcast m-row variants;
                                   multiplied IN PLACE over the xb tile)
A slice of the 16 z1 chunks is built on the otherwise-idle GpSimd (Pool)
engine to offload the Vector engine. GEMMs run on the Tensor engine in
fp16, layer-0 exploiting z0 symmetry (5 chunks of 128 contraction rows
instead of 8). relu+bias runs on the Scalar engine; the per-batch
d-reduction runs as one windowed tensor_reduce per layer per chunk on the
Vector engine (replacing 32 tiny accum activations). The final
(channel, batch) -> (batch, channel) transpose runs on the Tensor engine.
The chunk pipeline is software-pipelined two deep so every engine stays
busy; input tiles prefetch one chunk ahead on the Sync + Scalar DMA queues.
"""

import sys

sys.path.insert(0, "/opt/trn_rl_repo")

import numpy as np

import concourse.bass as bass
import concourse.mybir as mybir
import concourse.tile as tile
from concourse import bacc
from concourse.bass_utils import run_bass_kernel_spmd
from concourse.masks import make_identity

# ---- problem constants (hardcoded per contract) ---------------------------- #
B = 2048
F = 32  # field size (channels in)
D = 64  # embedding dim
O0 = 128  # layer-0 out channels
O1 = 128  # layer-1 out channels
H1 = 64  # split half fed to layer 1
NCORES = 8
BC = B // NCORES  # batch per core
NCHUNK = 1024  # GEMM columns per chunk (16 batch x 64 d)
BPC = NCHUNK // D  # batch elems per chunk
NCHUNKS = BC * D // NCHUNK
L0C = 5  # layer-0 z chunks (symmetric cover: difference classes 0..16)
L0_SHIFT = (0, 4, 8, 12, 16)  # per-chunk lane shift for the z0 row map
L1C = 16  # layer-1 z chunks (2048 rows / 128)
MMF = 512  # matmul free-dim per instruction
DT = mybir.dt.float16
FP32 = mybir.dt.float32

# ---- schedule knobs -------------------------------------------------------- #
# layer-1 chunk assignment: chunks in DVE_GROUPS run on the Vector engine
# (grouped ops, 2x mode); POOL_CHUNKS run on GpSimd.
DVE_GROUPS = ((0, 4), (4, 7), (7, 10))
POOL_CHUNKS = tuple(range(10, 16))
# xb prefetch DMA issue engine per chunk: first half via scalar queue,
# second half via sync queue (xa+xr0 also ride the sync queue).
XB_SPLIT = 8  # chunks < XB_SPLIT issue from scalar queue, rest from sync

# ---- host-side row maps ---------------------------------------------------- #
# z0 chunk c, row 32q+l  holds pair {h, m} = {(l + L0_SHIFT[c] + q) % 32, l}
# z1 chunk c, row 32q+l  holds pair (g, m) = (32*(q%2) + l, (c + 16*(q//2)) % 32)

_Q = np.arange(128) // 32
_L = np.arange(128) % 32
ROWS_XA = _L.copy()  # xa row map (x replicated per quadrant)
ROWS_XR0 = np.empty((128, L0C), dtype=np.int64)
for _c in range(L0C):
    ROWS_XR0[:, _c] = (_L + L0_SHIFT[_c] + _Q) % 32
ROWS_XB = np.empty((128, L1C), dtype=np.int64)
for _c in range(L1C):
    ROWS_XB[:, _c] = (_c + 16 * (_Q // 2)) % 32


def _prep_weights(W0, b0, W1, b1):
    w0 = W0.reshape(O0, F, F)  # [o, h, m]
    w0sym = w0 + w0.transpose(0, 2, 1)
    w0t = np.zeros((L0C, 128, O0), dtype=np.float16)
    for c in range(L0C):
        for q in range(4):
            delta = L0_SHIFT[c] + q
            if delta > 16:
                continue  # duplicate class, keep zero weights
            for l in range(32):
                if delta == 16 and l >= 16:
                    continue  # delta=16 pairs appear twice; keep first half
                h = (l + delta) % 32
                if delta == 0:
                    w0t[c, 32 * q + l, :] = w0[:, l, l].astype(np.float16)
                else:
                    w0t[c, 32 * q + l, :] = w0sym[:, h, l].astype(np.float16)
    w1 = W1.reshape(O1, H1, F)  # [o, g, m]
    w1t = np.empty((L1C, 128, O1), dtype=np.float16)
    for c in range(L1C):
        for q in range(4):
            m = (c + 16 * (q // 2)) % 32
            gbase = 32 * (q % 2)
            w1t[c, 32 * q : 32 * q + 32, :] = w1[:, gbase : gbase + 32, m].T.astype(
                np.float16
            )
    # [p, c, o] layout for contiguous per-partition DMA
    return (
        np.ascontiguousarray(w0t.transpose(1, 0, 2)),
        np.ascontiguousarray(w1t.transpose(1, 0, 2)),
        b0.astype(np.float32),
        b1.astype(np.float32),
    )


def _prep_inputs_core(x_core):
    """x_core: (BC, F, D) fp32 -> xa (N,128,K), xr0 (N,128,L0C,K),
    xb (N,128,L1C,K) fp16 operand tiles."""
    xcols = (
        x_core.reshape(NCHUNKS, BPC, F, D)
        .transpose(2, 0, 1, 3)
        .reshape(F, NCHUNKS, NCHUNK)
        .astype(np.float16)
    )
    xa = np.ascontiguousarray(xcols[ROWS_XA].transpose(1, 0, 2))
    xr0 = np.ascontiguousarray(xcols[ROWS_XR0].transpose(2, 0, 1, 3))
    xb = np.ascontiguousarray(xcols[ROWS_XB].transpose(2, 0, 1, 3))
    return xa, xr0, xb


# ---- kernel build ---------------------------------------------------------- #

_NC_CACHE = {}


def _build():
    nc = bacc.Bacc("TRN2", target_bir_lowering=False, debug=False)

    xa_d = nc.dram_tensor("xa", [NCHUNKS, 128, NCHUNK], DT, kind="ExternalInput")
    xr0_d = nc.dram_tensor(
        "xr0", [NCHUNKS, 128, L0C, NCHUNK], DT, kind="ExternalInput"
    )
    xb_d = nc.dram_tensor("xb", [NCHUNKS, 128, L1C, NCHUNK], DT, kind="ExternalInput")
    w0t_d = nc.dram_tensor("w0t", [128, L0C, O0], DT, kind="ExternalInput")
    w1t_d = nc.dram_tensor("w1t", [128, L1C, O1], DT, kind="ExternalInput")
    b0_d = nc.dram_tensor("b0", [O0, 1], FP32, kind="ExternalInput")
    b1_d = nc.dram_tensor("b1", [O1, 1], FP32, kind="ExternalInput")
    out_d = nc.dram_tensor("out", [BC, 192], FP32, kind="ExternalOutput")

    mult = mybir.AluOpType.mult

    with tile.TileContext(nc) as tc:
        with (
            tc.tile_pool(name="const", bufs=1) as cpool,
            tc.tile_pool(name="xin", bufs=2) as xpool,
            tc.tile_pool(name="xb", bufs=2) as xbpool,
            tc.tile_pool(name="z0", bufs=2) as z0pool,
            tc.tile_pool(name="y", bufs=2) as ypool,
            tc.tile_pool(name="f0", bufs=5) as f0pool,
            tc.tile_pool(name="f1", bufs=4) as f1pool,
            tc.tile_pool(name="psum", bufs=2, space="PSUM") as pspool,
        ):
            # resident weights, biases, accumulators, identity
            w0t = cpool.tile([128, L0C, O0], DT, tag="w0t")
            w1t = cpool.tile([128, L1C, O1], DT, tag="w1t")
            nc.sync.dma_start(w0t[:], w0t_d.ap())
            nc.sync.dma_start(w1t[:], w1t_d.ap())
            b0t = cpool.tile([O0, 1], FP32, tag="b0")
            b1t = cpool.tile([O1, 1], FP32, tag="b1")
            nc.sync.dma_start(b0t[:], b0_d.ap())
            nc.sync.dma_start(b1t[:], b1_d.ap())
            ident = cpool.tile([128, 128], FP32, tag="ident")
            make_identity(nc, ident[:])
            r0all = cpool.tile([128, BC], FP32, tag="r0all")  # rows 64:128 used
            r1all = cpool.tile([128, BC], FP32, tag="r1all")

            st = {}  # per-chunk live tiles

            def emit_dma(i):
                xa = xpool.tile([128, NCHUNK], DT, tag="xa", name=f"xa_{i}")
                nc.sync.dma_start(xa[:], xa_d.ap()[i])
                xr0 = xpool.tile([128, L0C, NCHUNK], DT, tag="xr0", name=f"xr0_{i}")
                nc.sync.dma_start(xr0[:], xr0_d.ap()[i])
                xb = xbpool.tile([128, L1C, NCHUNK], DT, tag="xb", name=f"xb_{i}")
                for c in range(L1C):
                    eng = nc.scalar if c < XB_SPLIT else nc.sync
                    eng.dma_start(xb[:, c], xb_d.ap()[i][:, c])
                st[i] = {"xa": xa, "xr0": xr0, "xb": xb}

            def emit_l0a(i):
                s = st[i]
                xa, xr0 = s["xa"], s["xr0"]
                z0 = z0pool.tile([128, L0C, NCHUNK], DT, tag="z0", name=f"z0_{i}")
                xa_b = xa[:].unsqueeze(1).broadcast_to((128, L0C, NCHUNK))
                nc.vector.tensor_tensor(z0[:], xr0[:], xa_b, op=mult)
                ps0 = pspool.tile([128, NCHUNK], FP32, tag="ps0", name=f"ps0_{i}")
                for c in range(L0C):
                    for sl in range(NCHUNK // MMF):
                        nc.tensor.matmul(
                            ps0[:, sl * MMF : (sl + 1) * MMF],
                            w0t[:, c],
                            z0[:, c, sl * MMF : (sl + 1) * MMF],
                            start=(c == 0),
                            stop=(c == L0C - 1),
                        )
                y2d = ypool.tile([128, NCHUNK], DT, tag="y2d", name=f"y2d_{i}")
                nc.scalar.activation(
                    y2d[:H1],
                    ps0[:H1],
                    mybir.ActivationFunctionType.Relu,
                    bias=b0t[:H1],
                )
                f0s = f0pool.tile([128, BPC, D], DT, tag="f0s", name=f"f0s_{i}")
                nc.scalar.activation(
                    f0s[H1:128].rearrange("p a b -> p (a b)"),
                    ps0[H1:128],
                    mybir.ActivationFunctionType.Relu,
                    bias=b0t[H1:128],
                )
                s["z0"] = z0
                s["ps0"] = ps0
                s["y2d"] = y2d
                s["f0s"] = f0s

            def emit_l0b(i):
                # duplicate y into the upper half for the z1 multiplies;
                # emitted after z1(i-1) so the Vector engine never stalls here
                y2d = st[i]["y2d"]
                nc.vector.tensor_copy(y2d[H1:128], y2d[:H1])

            def emit_l1(i):
                s = st[i]
                xb, y2d = s["xb"], s["y2d"]
                for a, b in DVE_GROUPS:
                    y_b = y2d[:].unsqueeze(1).broadcast_to((128, b - a, NCHUNK))
                    nc.vector.tensor_tensor(xb[:, a:b], xb[:, a:b], y_b, op=mult)
                for c in POOL_CHUNKS:
                    nc.gpsimd.tensor_tensor(xb[:, c], xb[:, c], y2d[:], op=mult)
                ps1 = pspool.tile([128, NCHUNK], FP32, tag="ps1", name=f"ps1_{i}")
                for c in range(L1C):
                    for sl in range(NCHUNK // MMF):
                        nc.tensor.matmul(
                            ps1[:, sl * MMF : (sl + 1) * MMF],
                            w1t[:, c],
                            xb[:, c, sl * MMF : (sl + 1) * MMF],
                            start=(c == 0),
                            stop=(c == L1C - 1),
                        )
                f1s = f1pool.tile([128, BPC, D], DT, tag="f1s", name=f"f1s_{i}")
                nc.scalar.activation(
                    f1s[:].rearrange("p a b -> p (a b)"),
                    ps1[:],
                    mybir.ActivationFunctionType.Relu,
                    bias=b1t[:],
                )
                s["f1s"] = f1s

            def emit_reds(i):
                s = st[i]
                cols = slice(i * BPC, (i + 1) * BPC)
                nc.vector.tensor_reduce(
                    r1all[:, cols],
                    s["f1s"][:],
                    axis=mybir.AxisListType.X,
                    op=mybir.AluOpType.add,
                )
                nc.vector.tensor_reduce(
                    r0all[H1:128, cols],
                    s["f0s"][H1:128],
                    axis=mybir.AxisListType.X,
                    op=mybir.AluOpType.add,
                )
                del st[i]

            for p in range(NCHUNKS + 5):
                if p < NCHUNKS:
                    emit_dma(p)
                if 1 <= p and p - 1 < NCHUNKS:
                    emit_l0a(p - 1)
                if 2 <= p and p - 2 < NCHUNKS:
                    emit_l1(p - 2)
                if 1 <= p and p - 1 < NCHUNKS:
                    emit_l0b(p - 1)
                if 4 <= p and p - 4 < NCHUNKS:
                    emit_reds(p - 4)

            # ---- final transpose (channel, batch) -> (batch, channel) ----
            outbuf = cpool.tile([128, 2, 192], FP32, tag="outbuf")
            for seg in range(2):
                cs = slice(seg * 128, (seg + 1) * 128)
                pt1 = pspool.tile([128, 128], FP32, tag="ps1", name=f"pt1_{seg}")
                nc.tensor.matmul(
                    pt1[:], r1all[:, cs], ident[:], is_transpose=True
                )
                nc.scalar.activation(
                    outbuf[:, seg, H1:192],
                    pt1[:],
                    mybir.ActivationFunctionType.Copy,
                )
                pt0 = pspool.tile([128, 64], FP32, tag="ps0", name=f"pt0_{seg}")
                nc.tensor.matmul(
                    pt0[:],
                    r0all[H1:128, cs],
                    ident[H1:128, H1:128],
                    is_transpose=True,
                )
                nc.scalar.activation(
                    outbuf[:, seg, 0:H1],
                    pt0[:],
                    mybir.ActivationFunctionType.Copy,
                )
            nc.sync.dma_start(
                out_d.ap().rearrange("(s b) c -> b s c", s=2), outbuf[:]
            )

    nc.compile()
    return nc


def _get_nc():
    if "nc" not in _NC_CACHE:
        _NC_CACHE["nc"] = _build()
    return _NC_CACHE["nc"]


def _make_in_maps(inputs, W0, b0, W1, b1):
    w0t, w1t, b0f, b1f = _prep_weights(W0, b0, W1, b1)
    in_maps = []
    for core in range(NCORES):
        xa, xr0, xb = _prep_inputs_core(inputs[core * BC : (core + 1) * BC])
        in_maps.append(
            {
                "xa": xa,
                "xr0": xr0,
                "xb": xb,
                "w0t": w0t,
                "w1t": w1t,
                "b0": b0f[:, None],
                "b1": b1f[:, None],
            }
        )
    return in_maps


def kernel(inputs, W0, b0, W1, b1):
    inputs = np.asarray(inputs, dtype=np.float32)
    nc = _get_nc()
    in_maps = _make_in_maps(
        inputs,
        np.asarray(W0, np.float32),
        np.asarray(b0, np.float32),
        np.asarray(W1, np.float32),
        np.asarray(b1, np.float32),
    )
    res = run_bass_kernel_spmd(nc, in_maps, core_ids=list(range(NCORES)))
    out = np.concatenate([res.results[c]["out"] for c in range(NCORES)], axis=0)
    return out.astype(np.float32)


def _install_ntff_hook():
    """The container's antenv lacks axon_hooks; synthesize it around the
    injected libaxon_pjrt.so so run_bass_kernel_spmd(trace=True) works."""
    import types

    if "antenv.axon_hooks" in sys.modules:
        return
    sys.path.insert(0, "/root/.axon_site")
    from trn_agent_boot.trn_boot import _ntff_profile_via_ctypes

    hook = _ntff_profile_via_ctypes("/opt/axon/libaxon_pjrt.so")
    m = types.ModuleType("antenv.axon_hooks")
    m.get_axon_ntff_profile_hook = lambda: hook
    m.set_axon_ntff_profile_hook = lambda h: None
    sys.modules["antenv.axon_hooks"] = m


def profile_once(inputs_dict, tmpdir=None):
    """Run once with NTFF tracing; return exec_time_ns (core 0)."""
    _install_ntff_hook()
    nc = _get_nc()
    in_maps = _make_in_maps(
        np.asarray(inputs_dict["inputs"], np.float32),
        np.asarray(inputs_dict["W0"], np.float32),
        np.asarray(inputs_dict["b0"], np.float32),
        np.asarray(inputs_dict["W1"], np.float32),
        np.asarray(inputs_dict["b1"], np.float32),
    )
    res = run_bass_kernel_spmd(
        nc, in_maps, core_ids=list(range(NCORES)), trace=True, tmpdir=tmpdir
    )
    return res.exec_time_ns


if __name__ == "__main__":
    rng = np.random.default_rng(0)
    inputs = rng.standard_normal((B, F, D), dtype=np.float32)
    W0 = (rng.standard_normal((O0, F * F), dtype=np.float32) * 0.03).astype(np.float32)
    W1 = (rng.standard_normal((O1, H1 * F), dtype=np.float32) * 0.03).astype(np.float32)
    b0 = np.zeros(O0, np.float32)
    b1 = np.zeros(O1, np.float32)
    out = kernel(inputs=inputs, W0=W0, b0=b0, W1=W1, b1=b1)
    print("kernel out", out.shape, out.dtype, out[:2, :4])


# revision 6
# speedup vs baseline: 1.4099x; 1.0539x over previous
"""Trainium2 Bass kernel for CIN (Compressed Interaction Network) forward.

Reference computation (per batch b, per dim d, with x = inputs[b, :, d], F=32):
  z0[(h,m)] = x[h]*x[m]                    (1024-vector)
  y0 = relu(W0 @ z0 + b0)                  (128)
  h1 = y0[:64]; f0 = y0[64:]
  z1[(g,m)] = h1[g]*x[m]                   (2048-vector)
  f1 = relu(W1 @ z1 + b1)                  (128)
  out[b, 0:64]  = sum_d f0
  out[b, 64:192] = sum_d f1

Strategy: pure data parallel over 8 cores (256 batch each). Per core the
(b, d) pairs form 16384 GEMM columns, processed in 16 chunks of 1024.

The outer-product feature tiles z are built with PLAIN tensor_tensor
multiplies (which run in the DVE's 2x packed-fp16 mode, 2 elem/lane/cycle)
against host-precomputed partition-rotated / partition-broadcast operand
tiles streamed from HBM:
  z0 chunk c: xr0[:, c] * xa      (xr0 = 5 host-rotated variants of x)
  z1 chunk c: xb[:, c] * y2d      (xb = 16 host-broadcast m-row variants;
                                   multiplied IN PLACE over the xb tile)
A slice of the 16 z1 chunks is built on the otherwise-idle GpSimd (Pool)
engine to offload the Vector engine. GEMMs run on the Tensor engine in
fp16, layer-0 exploiting z0 symmetry (5 chunks of 128 contraction rows
instead of 8). relu+bias runs on the Scalar engine; the per-batch
d-reduction runs as one windowed tensor_reduce per layer per chunk on the
Vector engine (replacing 32 tiny accum activations). The final
(channel, batch) -> (batch, channel) transpose runs on the Tensor engine.
The chunk pipeline is software-pipelined two deep so every engine stays
busy; input tiles prefetch one chunk ahead on the Sync + Scalar DMA queues.
"""

import sys

sys.path.insert(0, "/opt/trn_rl_repo")

import numpy as np

import concourse.bass as bass
import concourse.mybir as mybir
import concourse.tile as tile
from concourse import bacc
from concourse.bass_utils import run_bass_kernel_spmd
from concourse.masks import make_identity

# ---- problem constants (hardcoded per contract) ---------------------------- #
B = 2048
F = 32  # field size (channels in)
D = 64  # embedding dim
O0 = 128  # layer-0 out channels
O1 = 128  # layer-1 out channels
H1 = 64  # split half fed to layer 1
NCORES = 8
BC = B // NCORES  # batch per core
NCHUNK = 1024  # GEMM columns per chunk (16 batch x 64 d)
BPC = NCHUNK // D  # batch elems per chunk
NCHUNKS = BC * D // NCHUNK
L0C = 5  # layer-0 z chunks (symmetric cover: difference classes 0..16)
L0_SHIFT = (0, 4, 8, 12, 16)  # per-chunk lane shift for the z0 row map
L1C = 16  # layer-1 z chunks (2048 rows / 128)
MMF = 512  # matmul free-dim per instruction
DT = mybir.dt.float16
FP32 = mybir.dt.float32

# ---- schedule knobs -------------------------------------------------------- #
# layer-1 chunk assignment: chunks in DVE_GROUPS run on the Vector engine
# (grouped ops, 2x mode); POOL_CHUNKS run on GpSimd.
DVE_GROUPS = ((0, 4), (4, 8), (8, 11))
POOL_CHUNKS = tuple(range(11, 16))
# xb prefetch DMA issue engine per chunk: first half via scalar queue,
# second half via sync queue (xa+xr0 also ride the sync queue).
XB_SPLIT = 8  # chunks < XB_SPLIT issue from scalar queue, rest from sync

# ---- host-side row maps ---------------------------------------------------- #
# z0 chunk c, row 32q+l  holds pair {h, m} = {(l + L0_SHIFT[c] + q) % 32, l}
# z1 chunk c, row 32q+l  holds pair (g, m) = (32*(q%2) + l, (c + 16*(q//2)) % 32)

_Q = np.arange(128) // 32
_L = np.arange(128) % 32
ROWS_XA = _L.copy()  # xa row map (x replicated per quadrant)
ROWS_XR0 = np.empty((128, L0C), dtype=np.int64)
for _c in range(L0C):
    ROWS_XR0[:, _c] = (_L + L0_SHIFT[_c] + _Q) % 32
ROWS_XB = np.empty((128, L1C), dtype=np.int64)
for _c in range(L1C):
    ROWS_XB[:, _c] = (_c + 16 * (_Q // 2)) % 32


def _prep_weights(W0, b0, W1, b1):
    w0 = W0.reshape(O0, F, F)  # [o, h, m]
    w0sym = w0 + w0.transpose(0, 2, 1)
    w0t = np.zeros((L0C, 128, O0), dtype=np.float16)
    for c in range(L0C):
        for q in range(4):
            delta = L0_SHIFT[c] + q
            if delta > 16:
                continue  # duplicate class, keep zero weights
            for l in range(32):
                if delta == 16 and l >= 16:
                    continue  # delta=16 pairs appear twice; keep first half
                h = (l + delta) % 32
                if delta == 0:
                    w0t[c, 32 * q + l, :] = w0[:, l, l].astype(np.float16)
                else:
                    w0t[c, 32 * q + l, :] = w0sym[:, h, l].astype(np.float16)
    w1 = W1.reshape(O1, H1, F)  # [o, g, m]
    w1t = np.empty((L1C, 128, O1), dtype=np.float16)
    for c in range(L1C):
        for q in range(4):
            m = (c + 16 * (q // 2)) % 32
            gbase = 32 * (q % 2)
            w1t[c, 32 * q : 32 * q + 32, :] = w1[:, gbase : gbase + 32, m].T.astype(
                np.float16
            )
    # [p, c, o] layout for contiguous per-partition DMA
    return (
        np.ascontiguousarray(w0t.transpose(1, 0, 2)),
        np.ascontiguousarray(w1t.transpose(1, 0, 2)),
        b0.astype(np.float32),
        b1.astype(np.float32),
    )


def _prep_inputs_core(x_core):
    """x_core: (BC, F, D) fp32 -> xa (N,128,K), xr0 (N,128,L0C,K),
    xb (N,128,L1C,K) fp16 operand tiles."""
    xcols = (
        x_core.reshape(NCHUNKS, BPC, F, D)
        .transpose(2, 0, 1, 3)
        .reshape(F, NCHUNKS, NCHUNK)
        .astype(np.float16)
    )
    xa = np.ascontiguousarray(xcols[ROWS_XA].transpose(1, 0, 2))
    xr0 = np.ascontiguousarray(xcols[ROWS_XR0].transpose(2, 0, 1, 3))
    xb = np.ascontiguousarray(xcols[ROWS_XB].transpose(2, 0, 1, 3))
    return xa, xr0, xb


# ---- kernel build ---------------------------------------------------------- #

_NC_CACHE = {}


def _build():
    nc = bacc.Bacc("TRN2", target_bir_lowering=False, debug=False)

    xa_d = nc.dram_tensor("xa", [NCHUNKS, 128, NCHUNK], DT, kind="ExternalInput")
    xr0_d = nc.dram_tensor(
        "xr0", [NCHUNKS, 128, L0C, NCHUNK], DT, kind="ExternalInput"
    )
    xb_d = nc.dram_tensor("xb", [NCHUNKS, 128, L1C, NCHUNK], DT, kind="ExternalInput")
    w0t_d = nc.dram_tensor("w0t", [128, L0C, O0], DT, kind="ExternalInput")
    w1t_d = nc.dram_tensor("w1t", [128, L1C, O1], DT, kind="ExternalInput")
    b0_d = nc.dram_tensor("b0", [O0, 1], FP32, kind="ExternalInput")
    b1_d = nc.dram_tensor("b1", [O1, 1], FP32, kind="ExternalInput")
    out_d = nc.dram_tensor("out", [BC, 192], FP32, kind="ExternalOutput")

    mult = mybir.AluOpType.mult

    with tile.TileContext(nc) as tc:
        with (
            tc.tile_pool(name="const", bufs=1) as cpool,
            tc.tile_pool(name="xin", bufs=2) as xpool,
            tc.tile_pool(name="xb", bufs=3) as xbpool,
            tc.tile_pool(name="z0", bufs=2) as z0pool,
            tc.tile_pool(name="y", bufs=2) as ypool,
            tc.tile_pool(name="f0", bufs=5) as f0pool,
            tc.tile_pool(name="f1", bufs=4) as f1pool,
            tc.tile_pool(name="psum", bufs=2, space="PSUM") as pspool,
        ):
            # resident weights, biases, accumulators, identity
            w0t = cpool.tile([128, L0C, O0], DT, tag="w0t")
            w1t = cpool.tile([128, L1C, O1], DT, tag="w1t")
            nc.sync.dma_start(w0t[:], w0t_d.ap())
            nc.sync.dma_start(w1t[:], w1t_d.ap())
            b0t = cpool.tile([O0, 1], FP32, tag="b0")
            b1t = cpool.tile([O1, 1], FP32, tag="b1")
            nc.sync.dma_start(b0t[:], b0_d.ap())
            nc.sync.dma_start(b1t[:], b1_d.ap())
            ident = cpool.tile([128, 128], FP32, tag="ident")
            make_identity(nc, ident[:])
            r0all = cpool.tile([128, BC], FP32, tag="r0all")  # rows 64:128 used
            r1all = cpool.tile([128, BC], FP32, tag="r1all")

            st = {}  # per-chunk live tiles

            def emit_dma(i):
                xa = xpool.tile([128, NCHUNK], DT, tag="xa", name=f"xa_{i}")
                nc.sync.dma_start(xa[:], xa_d.ap()[i])
                xr0 = xpool.tile([128, L0C, NCHUNK], DT, tag="xr0", name=f"xr0_{i}")
                nc.sync.dma_start(xr0[:], xr0_d.ap()[i])
                xb = xbpool.tile([128, L1C, NCHUNK], DT, tag="xb", name=f"xb_{i}")
                for c in range(L1C):
                    eng = nc.scalar if c < XB_SPLIT else nc.sync
                    eng.dma_start(xb[:, c], xb_d.ap()[i][:, c])
                st[i] = {"xa": xa, "xr0": xr0, "xb": xb}

            def emit_l0a(i):
                s = st[i]
                xa, xr0 = s["xa"], s["xr0"]
                z0 = z0pool.tile([128, L0C, NCHUNK], DT, tag="z0", name=f"z0_{i}")
                xa_b = xa[:].unsqueeze(1).broadcast_to((128, L0C, NCHUNK))
                nc.vector.tensor_tensor(z0[:], xr0[:], xa_b, op=mult)
                ps0 = pspool.tile([128, NCHUNK], FP32, tag="ps0", name=f"ps0_{i}")
                for c in range(L0C):
                    for sl in range(NCHUNK // MMF):
                        nc.tensor.matmul(
                            ps0[:, sl * MMF : (sl + 1) * MMF],
                            w0t[:, c],
                            z0[:, c, sl * MMF : (sl + 1) * MMF],
                            start=(c == 0),
                            stop=(c == L0C - 1),
                        )
                y2d = ypool.tile([128, NCHUNK], DT, tag="y2d", name=f"y2d_{i}")
                nc.scalar.activation(
                    y2d[:H1],
                    ps0[:H1],
                    mybir.ActivationFunctionType.Relu,
                    bias=b0t[:H1],
                )
                f0s = f0pool.tile([128, BPC, D], DT, tag="f0s", name=f"f0s_{i}")
                nc.scalar.activation(
                    f0s[H1:128].rearrange("p a b -> p (a b)"),
                    ps0[H1:128],
                    mybir.ActivationFunctionType.Relu,
                    bias=b0t[H1:128],
                )
                s["z0"] = z0
                s["ps0"] = ps0
                s["y2d"] = y2d
                s["f0s"] = f0s

            def emit_l0b(i):
                # duplicate y into the upper half for the z1 multiplies, on
                # the Scalar engine (partition-shifted copy; DVE stays free)
                y2d = st[i]["y2d"]
                nc.scalar.activation(
                    y2d[H1:128], y2d[:H1], mybir.ActivationFunctionType.Copy
                )

            def emit_l1(i):
                s = st[i]
                xb, y2d = s["xb"], s["y2d"]
                for a, b in DVE_GROUPS:
                    y_b = y2d[:].unsqueeze(1).broadcast_to((128, b - a, NCHUNK))
                    nc.vector.tensor_tensor(xb[:, a:b], xb[:, a:b], y_b, op=mult)
                for c in POOL_CHUNKS:
                    nc.gpsimd.tensor_tensor(xb[:, c], xb[:, c], y2d[:], op=mult)
                ps1 = pspool.tile([128, NCHUNK], FP32, tag="ps1", name=f"ps1_{i}")
                for c in range(L1C):
                    for sl in range(NCHUNK // MMF):
                        nc.tensor.matmul(
                            ps1[:, sl * MMF : (sl + 1) * MMF],
                            w1t[:, c],
                            xb[:, c, sl * MMF : (sl + 1) * MMF],
                            start=(c == 0),
                            stop=(c == L1C - 1),
                        )
                f1s = f1pool.tile([128, BPC, D], DT, tag="f1s", name=f"f1s_{i}")
                nc.scalar.activation(
                    f1s[:].rearrange("p a b -> p (a b)"),
                    ps1[:],
                    mybir.ActivationFunctionType.Relu,
                    bias=b1t[:],
                )
                s["f1s"] = f1s

            def emit_reds(i):
                s = st[i]
                cols = slice(i * BPC, (i + 1) * BPC)
                nc.vector.tensor_reduce(
                    r1all[:, cols],
                    s["f1s"][:],
                    axis=mybir.AxisListType.X,
                    op=mybir.AluOpType.add,
                )
                nc.vector.tensor_reduce(
                    r0all[H1:128, cols],
                    s["f0s"][H1:128],
                    axis=mybir.AxisListType.X,
                    op=mybir.AluOpType.add,
                )
                del st[i]

            for p in range(NCHUNKS + 5):
                if p < NCHUNKS:
                    emit_dma(p)
                if 1 <= p and p - 1 < NCHUNKS:
                    emit_l0a(p - 1)
                    emit_l0b(p - 1)
                if 2 <= p and p - 2 < NCHUNKS:
                    emit_l1(p - 2)
                if 4 <= p and p - 4 < NCHUNKS:
                    emit_reds(p - 4)

            # ---- final transpose (channel, batch) -> (batch, channel) ----
            outbuf = cpool.tile([128, 2, 192], FP32, tag="outbuf")
            for seg in range(2):
                cs = slice(seg * 128, (seg + 1) * 128)
                pt1 = pspool.tile([128, 128], FP32, tag="ps1", name=f"pt1_{seg}")
                nc.tensor.matmul(
                    pt1[:], r1all[:, cs], ident[:], is_transpose=True
                )
                nc.scalar.activation(
                    outbuf[:, seg, H1:192],
                    pt1[:],
                    mybir.ActivationFunctionType.Copy,
                )
                pt0 = pspool.tile([128, 64], FP32, tag="ps0", name=f"pt0_{seg}")
                nc.tensor.matmul(
                    pt0[:],
                    r0all[H1:128, cs],
                    ident[H1:128, H1:128],
                    is_transpose=True,
                )
                nc.scalar.activation(
                    outbuf[:, seg, 0:H1],
                    pt0[:],
                    mybir.ActivationFunctionType.Copy,
                )
            nc.sync.dma_start(
                out_d.ap().rearrange("(s b) c -> b s c", s=2), outbuf[:]
            )

    nc.compile()
    return nc


def _get_nc():
    if "nc" not in _NC_CACHE:
        _NC_CACHE["nc"] = _build()
    return _NC_CACHE["nc"]


def _make_in_maps(inputs, W0, b0, W1, b1):
    w0t, w1t, b0f, b1f = _prep_weights(W0, b0, W1, b1)
    in_maps = []
    for core in range(NCORES):
        xa, xr0, xb = _prep_inputs_core(inputs[core * BC : (core + 1) * BC])
        in_maps.append(
            {
                "xa": xa,
                "xr0": xr0,
                "xb": xb,
                "w0t": w0t,
                "w1t": w1t,
                "b0": b0f[:, None],
                "b1": b1f[:, None],
            }
        )
    return in_maps


def kernel(inputs, W0, b0, W1, b1):
    inputs = np.asarray(inputs, dtype=np.float32)
    nc = _get_nc()
    in_maps = _make_in_maps(
        inputs,
        np.asarray(W0, np.float32),
        np.asarray(b0, np.float32),
        np.asarray(W1, np.float32),
        np.asarray(b1, np.float32),
    )
    res = run_bass_kernel_spmd(nc, in_maps, core_ids=list(range(NCORES)))
    out = np.concatenate([res.results[c]["out"] for c in range(NCORES)], axis=0)
    return out.astype(np.float32)


def _install_ntff_hook():
    """The container's antenv lacks axon_hooks; synthesize it around the
    injected libaxon_pjrt.so so run_bass_kernel_spmd(trace=True) works."""
    import types

    if "antenv.axon_hooks" in sys.modules:
        return
    sys.path.insert(0, "/root/.axon_site")
    from trn_agent_boot.trn_boot import _ntff_profile_via_ctypes

    hook = _ntff_profile_via_ctypes("/opt/axon/libaxon_pjrt.so")
    m = types.ModuleType("antenv.axon_hooks")
    m.get_axon_ntff_profile_hook = lambda: hook
    m.set_axon_ntff_profile_hook = lambda h: None
    sys.modules["antenv.axon_hooks"] = m


def profile_once(inputs_dict, tmpdir=None):
    """Run once with NTFF tracing; return exec_time_ns (core 0)."""
    _install_ntff_hook()
    nc = _get_nc()
    in_maps = _make_in_maps(
        np.asarray(inputs_dict["inputs"], np.float32),
        np.asarray(inputs_dict["W0"], np.float32),
        np.asarray(inputs_dict["b0"], np.float32),
        np.asarray(inputs_dict["W1"], np.float32),
        np.asarray(inputs_dict["b1"], np.float32),
    )
    res = run_bass_kernel_spmd(
        nc, in_maps, core_ids=list(range(NCORES)), trace=True, tmpdir=tmpdir
    )
    return res.exec_time_ns


if __name__ == "__main__":
    rng = np.random.default_rng(0)
    inputs = rng.standard_normal((B, F, D), dtype=np.float32)
    W0 = (rng.standard_normal((O0, F * F), dtype=np.float32) * 0.03).astype(np.float32)
    W1 = (rng.standard_normal((O1, H1 * F), dtype=np.float32) * 0.03).astype(np.float32)
    b0 = np.zeros(O0, np.float32)
    b1 = np.zeros(O1, np.float32)
    out = kernel(inputs=inputs, W0=W0, b0=b0, W1=W1, b1=b1)
    print("kernel out", out.shape, out.dtype, out[:2, :4])


# revision 10
# speedup vs baseline: 1.4295x; 1.0139x over previous
"""Trainium2 Bass kernel for CIN (Compressed Interaction Network) forward.

Reference computation (per batch b, per dim d, with x = inputs[b, :, d], F=32):
  z0[(h,m)] = x[h]*x[m]                    (1024-vector)
  y0 = relu(W0 @ z0 + b0)                  (128)
  h1 = y0[:64]; f0 = y0[64:]
  z1[(g,m)] = h1[g]*x[m]                   (2048-vector)
  f1 = relu(W1 @ z1 + b1)                  (128)
  out[b, 0:64]  = sum_d f0
  out[b, 64:192] = sum_d f1

Strategy: pure data parallel over 8 cores (256 batch each). Per core the
(b, d) pairs form 16384 GEMM columns, processed in 16 chunks of 1024.

The outer-product feature tiles z are built with PLAIN tensor_tensor
multiplies (which run in the DVE's 2x packed-fp16 mode, 2 elem/lane/cycle)
against host-precomputed partition-rotated / partition-broadcast operand
tiles streamed from HBM:
  z0 chunk c: xr0[:, c] * xa      (xr0 = 5 host-rotated variants of x)
  z1 chunk c: xb[:, c] * y2d      (xb = 16 host-broadcast m-row variants;
                                   multiplied IN PLACE over the xb tile)
A slice of the 16 z1 chunks is built on the otherwise-idle GpSimd (Pool)
engine to offload the Vector engine. GEMMs run on the Tensor engine in
fp16, layer-0 exploiting z0 symmetry (5 chunks of 128 contraction rows
instead of 8). relu+bias runs on the Scalar engine; the per-batch
d-reduction runs as one windowed tensor_reduce per layer per chunk on the
Vector engine (replacing 32 tiny accum activations). The final
(channel, batch) -> (batch, channel) transpose runs on the Tensor engine.
The chunk pipeline is software-pipelined two deep so every engine stays
busy; input tiles prefetch one chunk ahead on the Sync + Scalar DMA queues.
"""

import sys

sys.path.insert(0, "/opt/trn_rl_repo")

import numpy as np

import concourse.bass as bass
import concourse.mybir as mybir
import concourse.tile as tile
from concourse import bacc
from concourse.bass_utils import run_bass_kernel_spmd
from concourse.masks import make_identity

# ---- problem constants (hardcoded per contract) ---------------------------- #
B = 2048
F = 32  # field size (channels in)
D = 64  # embedding dim
O0 = 128  # layer-0 out channels
O1 = 128  # layer-1 out channels
H1 = 64  # split half fed to layer 1
NCORES = 8
BC = B // NCORES  # batch per core
NCHUNK = 1024  # GEMM columns per chunk (16 batch x 64 d)
BPC = NCHUNK // D  # batch elems per chunk
NCHUNKS = BC * D // NCHUNK
L0C = 5  # layer-0 z chunks (symmetric cover: difference classes 0..16)
L0_SHIFT = (0, 4, 8, 12, 16)  # per-chunk lane shift for the z0 row map
L1C = 16  # layer-1 z chunks (2048 rows / 128)
MMF = 512  # matmul free-dim per instruction
DT = mybir.dt.float16
FP32 = mybir.dt.float32

# ---- schedule knobs -------------------------------------------------------- #
# layer-1 chunk assignment: chunks in DVE_GROUPS run on the Vector engine
# (grouped ops, 2x mode); POOL_CHUNKS run on GpSimd.
DVE_GROUPS = ((0, 4), (4, 8), (8, 11))
POOL_CHUNKS = tuple(range(11, 16))
# xb prefetch DMA issue engine per chunk: first half via scalar queue,
# second half via sync queue (xa+xr0 also ride the sync queue).
XB_SPLIT = 8  # chunks < XB_SPLIT issue from scalar queue, rest from sync

# ---- host-side row maps ---------------------------------------------------- #
# z0 chunk c, row 32q+l  holds pair {h, m} = {(l + L0_SHIFT[c] + q) % 32, l}
# z1 chunk c, row 32q+l  holds pair (g, m) = (32*(q%2) + l, (c + 16*(q//2)) % 32)

_Q = np.arange(128) // 32
_L = np.arange(128) % 32
ROWS_XA = _L.copy()  # xa row map (x replicated per quadrant)
ROWS_XR0 = np.empty((128, L0C), dtype=np.int64)
for _c in range(L0C):
    ROWS_XR0[:, _c] = (_L + L0_SHIFT[_c] + _Q) % 32
ROWS_XB = np.empty((128, L1C), dtype=np.int64)
for _c in range(L1C):
    ROWS_XB[:, _c] = (_c + 16 * (_Q // 2)) % 32


def _prep_weights(W0, b0, W1, b1):
    w0 = W0.reshape(O0, F, F)  # [o, h, m]
    w0sym = w0 + w0.transpose(0, 2, 1)
    w0t = np.zeros((L0C, 128, O0), dtype=np.float16)
    for c in range(L0C):
        for q in range(4):
            delta = L0_SHIFT[c] + q
            if delta > 16:
                continue  # duplicate class, keep zero weights
            for l in range(32):
                if delta == 16 and l >= 16:
                    continue  # delta=16 pairs appear twice; keep first half
                h = (l + delta) % 32
                if delta == 0:
                    w0t[c, 32 * q + l, :] = w0[:, l, l].astype(np.float16)
                else:
                    w0t[c, 32 * q + l, :] = w0sym[:, h, l].astype(np.float16)
    w1 = W1.reshape(O1, H1, F)  # [o, g, m]
    w1t = np.empty((L1C, 128, O1), dtype=np.float16)
    for c in range(L1C):
        for q in range(4):
            m = (c + 16 * (q // 2)) % 32
            gbase = 32 * (q % 2)
            w1t[c, 32 * q : 32 * q + 32, :] = w1[:, gbase : gbase + 32, m].T.astype(
                np.float16
            )
    # [p, c, o] layout for contiguous per-partition DMA
    return (
        np.ascontiguousarray(w0t.transpose(1, 0, 2)),
        np.ascontiguousarray(w1t.transpose(1, 0, 2)),
        b0.astype(np.float32),
        b1.astype(np.float32),
    )


def _prep_inputs_core(x_core):
    """x_core: (BC, F, D) fp32 -> xa (N,128,K), xr0 (N,128,L0C,K),
    xb (N,128,L1C,K) fp16 operand tiles."""
    xcols = (
        x_core.reshape(NCHUNKS, BPC, F, D)
        .transpose(2, 0, 1, 3)
        .reshape(F, NCHUNKS, NCHUNK)
        .astype(np.float16)
    )
    xa = np.ascontiguousarray(xcols[ROWS_XA].transpose(1, 0, 2))
    xr0 = np.ascontiguousarray(xcols[ROWS_XR0].transpose(2, 0, 1, 3))
    xb = np.ascontiguousarray(xcols[ROWS_XB].transpose(2, 0, 1, 3))
    return xa, xr0, xb


# ---- kernel build ---------------------------------------------------------- #

_NC_CACHE = {}


def _build():
    nc = bacc.Bacc("TRN2", target_bir_lowering=False, debug=False)

    xa_d = nc.dram_tensor("xa", [NCHUNKS, 128, NCHUNK], DT, kind="ExternalInput")
    xr0_d = nc.dram_tensor(
        "xr0", [NCHUNKS, 128, L0C, NCHUNK], DT, kind="ExternalInput"
    )
    xb_d = nc.dram_tensor("xb", [NCHUNKS, 128, L1C, NCHUNK], DT, kind="ExternalInput")
    w0t_d = nc.dram_tensor("w0t", [128, L0C, O0], DT, kind="ExternalInput")
    w1t_d = nc.dram_tensor("w1t", [128, L1C, O1], DT, kind="ExternalInput")
    b0_d = nc.dram_tensor("b0", [O0, 1], FP32, kind="ExternalInput")
    b1_d = nc.dram_tensor("b1", [O1, 1], FP32, kind="ExternalInput")
    out_d = nc.dram_tensor("out", [BC, 192], FP32, kind="ExternalOutput")

    mult = mybir.AluOpType.mult

    with tile.TileContext(nc) as tc:
        with (
            tc.tile_pool(name="const", bufs=1) as cpool,
            tc.tile_pool(name="xin", bufs=2) as xpool,
            tc.tile_pool(name="xb", bufs=3) as xbpool,
            tc.tile_pool(name="z0", bufs=2) as z0pool,
            tc.tile_pool(name="y", bufs=2) as ypool,
            tc.tile_pool(name="f0", bufs=3) as f0pool,
            tc.tile_pool(name="f1", bufs=2) as f1pool,
            tc.tile_pool(name="psum", bufs=2, space="PSUM") as pspool,
        ):
            # resident weights, biases, accumulators, identity
            w0t = cpool.tile([128, L0C, O0], DT, tag="w0t")
            w1t = cpool.tile([128, L1C, O1], DT, tag="w1t")
            nc.sync.dma_start(w0t[:], w0t_d.ap())
            nc.sync.dma_start(w1t[:], w1t_d.ap())
            b0t = cpool.tile([O0, 1], FP32, tag="b0")
            b1t = cpool.tile([O1, 1], FP32, tag="b1")
            nc.sync.dma_start(b0t[:], b0_d.ap())
            nc.sync.dma_start(b1t[:], b1_d.ap())
            ident = cpool.tile([128, 128], FP32, tag="ident")
            make_identity(nc, ident[:])
            r0all = cpool.tile([128, BC], FP32, tag="r0all")  # rows 64:128 used
            r1all = cpool.tile([128, BC], FP32, tag="r1all")

            st = {}  # per-chunk live tiles

            def emit_dma(i):
                xa = xpool.tile([128, NCHUNK], DT, tag="xa", name=f"xa_{i}")
                nc.sync.dma_start(xa[:], xa_d.ap()[i])
                xr0 = xpool.tile([128, L0C, NCHUNK], DT, tag="xr0", name=f"xr0_{i}")
                nc.sync.dma_start(xr0[:], xr0_d.ap()[i])
                xb = xbpool.tile([128, L1C, NCHUNK], DT, tag="xb", name=f"xb_{i}")
                for c in range(L1C):
                    eng = nc.scalar if c < XB_SPLIT else nc.sync
                    eng.dma_start(xb[:, c], xb_d.ap()[i][:, c])
                st[i] = {"xa": xa, "xr0": xr0, "xb": xb}

            def emit_l0a(i):
                s = st[i]
                xa, xr0 = s["xa"], s["xr0"]
                z0 = z0pool.tile([128, L0C, NCHUNK], DT, tag="z0", name=f"z0_{i}")
                xa_b = xa[:].unsqueeze(1).broadcast_to((128, L0C, NCHUNK))
                nc.vector.tensor_tensor(z0[:], xr0[:], xa_b, op=mult)
                ps0 = pspool.tile([128, NCHUNK], FP32, tag="ps0", name=f"ps0_{i}")
                for c in range(L0C):
                    for sl in range(NCHUNK // MMF):
                        nc.tensor.matmul(
                            ps0[:, sl * MMF : (sl + 1) * MMF],
                            w0t[:, c],
                            z0[:, c, sl * MMF : (sl + 1) * MMF],
                            start=(c == 0),
                            stop=(c == L0C - 1),
                        )
                y2d = ypool.tile([128, NCHUNK], DT, tag="y2d", name=f"y2d_{i}")
                nc.scalar.activation(
                    y2d[:H1],
                    ps0[:H1],
                    mybir.ActivationFunctionType.Relu,
                    bias=b0t[:H1],
                )
                f0s = f0pool.tile([128, BPC, D], DT, tag="f0s", name=f"f0s_{i}")
                nc.scalar.activation(
                    f0s[H1:128].rearrange("p a b -> p (a b)"),
                    ps0[H1:128],
                    mybir.ActivationFunctionType.Relu,
                    bias=b0t[H1:128],
                )
                s["z0"] = z0
                s["ps0"] = ps0
                s["y2d"] = y2d
                s["f0s"] = f0s

            def emit_l0b(i):
                # duplicate y into the upper half for the z1 multiplies, on
                # the Scalar engine (partition-shifted copy; DVE stays free)
                y2d = st[i]["y2d"]
                nc.scalar.activation(
                    y2d[H1:128], y2d[:H1], mybir.ActivationFunctionType.Copy
                )

            def emit_l1(i):
                s = st[i]
                xb, y2d = s["xb"], s["y2d"]
                for a, b in DVE_GROUPS:
                    y_b = y2d[:].unsqueeze(1).broadcast_to((128, b - a, NCHUNK))
                    nc.vector.tensor_tensor(xb[:, a:b], xb[:, a:b], y_b, op=mult)
                for c in POOL_CHUNKS:
                    nc.gpsimd.tensor_tensor(xb[:, c], xb[:, c], y2d[:], op=mult)
                ps1 = pspool.tile([128, NCHUNK], FP32, tag="ps1", name=f"ps1_{i}")
                for c in range(L1C):
                    for sl in range(NCHUNK // MMF):
                        nc.tensor.matmul(
                            ps1[:, sl * MMF : (sl + 1) * MMF],
                            w1t[:, c],
                            xb[:, c, sl * MMF : (sl + 1) * MMF],
                            start=(c == 0),
                            stop=(c == L1C - 1),
                        )
                f1s = f1pool.tile([128, BPC, D], DT, tag="f1s", name=f"f1s_{i}")
                nc.scalar.activation(
                    f1s[:].rearrange("p a b -> p (a b)"),
                    ps1[:],
                    mybir.ActivationFunctionType.Relu,
                    bias=b1t[:],
                )
                s["f1s"] = f1s

            def emit_red0(i):
                cols = slice(i * BPC, (i + 1) * BPC)
                nc.vector.tensor_reduce(
                    r0all[H1:128, cols],
                    st[i]["f0s"][H1:128],
                    axis=mybir.AxisListType.X,
                    op=mybir.AluOpType.add,
                )

            def emit_red1(i):
                cols = slice(i * BPC, (i + 1) * BPC)
                nc.vector.tensor_reduce(
                    r1all[:, cols],
                    st[i]["f1s"][:],
                    axis=mybir.AxisListType.X,
                    op=mybir.AluOpType.add,
                )
                del st[i]

            for p in range(NCHUNKS + 5):
                if p < NCHUNKS:
                    emit_dma(p)
                if 1 <= p and p - 1 < NCHUNKS:
                    emit_l0a(p - 1)
                    emit_l0b(p - 1)
                if 2 <= p and p - 2 < NCHUNKS:
                    emit_l1(p - 2)
                    emit_red0(p - 2)
                if 3 <= p and p - 3 < NCHUNKS:
                    emit_red1(p - 3)

            # ---- final transpose (channel, batch) -> (batch, channel) ----
            outbuf = cpool.tile([128, 2, 192], FP32, tag="outbuf")
            for seg in range(2):
                cs = slice(seg * 128, (seg + 1) * 128)
                pt1 = pspool.tile([128, 128], FP32, tag="ps1", name=f"pt1_{seg}")
                nc.tensor.matmul(
                    pt1[:], r1all[:, cs], ident[:], is_transpose=True
                )
                nc.scalar.activation(
                    outbuf[:, seg, H1:192],
                    pt1[:],
                    mybir.ActivationFunctionType.Copy,
                )
                pt0 = pspool.tile([128, 64], FP32, tag="ps0", name=f"pt0_{seg}")
                nc.tensor.matmul(
                    pt0[:],
                    r0all[H1:128, cs],
                    ident[H1:128, H1:128],
                    is_transpose=True,
                )
                nc.scalar.activation(
                    outbuf[:, seg, 0:H1],
                    pt0[:],
                    mybir.ActivationFunctionType.Copy,
                )
            nc.sync.dma_start(
                out_d.ap().rearrange("(s b) c -> b s c", s=2), outbuf[:]
            )

    nc.compile()
    return nc


def _get_nc():
    if "nc" not in _NC_CACHE:
        _NC_CACHE["nc"] = _build()
    return _NC_CACHE["nc"]


def _make_in_maps(inputs, W0, b0, W1, b1):
    w0t, w1t, b0f, b1f = _prep_weights(W0, b0, W1, b1)
    in_maps = []
    for core in range(NCORES):
        xa, xr0, xb = _prep_inputs_core(inputs[core * BC : (core + 1) * BC])
        in_maps.append(
            {
                "xa": xa,
                "xr0": xr0,
                "xb": xb,
                "w0t": w0t,
                "w1t": w1t,
                "b0": b0f[:, None],
                "b1": b1f[:, None],
            }
        )
    return in_maps


def kernel(inputs, W0, b0, W1, b1):
    inputs = np.asarray(inputs, dtype=np.float32)
    nc = _get_nc()
    in_maps = _make_in_maps(
        inputs,
        np.asarray(W0, np.float32),
        np.asarray(b0, np.float32),
        np.asarray(W1, np.float32),
        np.asarray(b1, np.float32),
    )
    res = run_bass_kernel_spmd(nc, in_maps, core_ids=list(range(NCORES)))
    out = np.concatenate([res.results[c]["out"] for c in range(NCORES)], axis=0)
    return out.astype(np.float32)


def _install_ntff_hook():
    """The container's antenv lacks axon_hooks; synthesize it around the
    injected libaxon_pjrt.so so run_bass_kernel_spmd(trace=True) works."""
    import types

    if "antenv.axon_hooks" in sys.modules:
        return
    sys.path.insert(0, "/root/.axon_site")
    from trn_agent_boot.trn_boot import _ntff_profile_via_ctypes

    hook = _ntff_profile_via_ctypes("/opt/axon/libaxon_pjrt.so")
    m = types.ModuleType("antenv.axon_hooks")
    m.get_axon_ntff_profile_hook = lambda: hook
    m.set_axon_ntff_profile_hook = lambda h: None
    sys.modules["antenv.axon_hooks"] = m


def profile_once(inputs_dict, tmpdir=None):
    """Run once with NTFF tracing; return exec_time_ns (core 0)."""
    _install_ntff_hook()
    nc = _get_nc()
    in_maps = _make_in_maps(
        np.asarray(inputs_dict["inputs"], np.float32),
        np.asarray(inputs_dict["W0"], np.float32),
        np.asarray(inputs_dict["b0"], np.float32),
        np.asarray(inputs_dict["W1"], np.float32),
        np.asarray(inputs_dict["b1"], np.float32),
    )
    res = run_bass_kernel_spmd(
        nc, in_maps, core_ids=list(range(NCORES)), trace=True, tmpdir=tmpdir
    )
    return res.exec_time_ns


if __name__ == "__main__":
    rng = np.random.default_rng(0)
    inputs = rng.standard_normal((B, F, D), dtype=np.float32)
    W0 = (rng.standard_normal((O0, F * F), dtype=np.float32) * 0.03).astype(np.float32)
    W1 = (rng.standard_normal((O1, H1 * F), dtype=np.float32) * 0.03).astype(np.float32)
    b0 = np.zeros(O0, np.float32)
    b1 = np.zeros(O1, np.float32)
    out = kernel(inputs=inputs, W0=W0, b0=b0, W1=W1, b1=b1)
    print("kernel out", out.shape, out.dtype, out[:2, :4])
